# revision 30
# baseline (speedup 1.0000x reference)
"""Trainium2 Bass kernel for nn_DiffusionModel (Sinkhorn OT assignment + per-point MLP).

Data-parallel over the batch: each of the 8 NeuronCores processes one sample
(B=8).  Per core, the cost matrix is never materialized: each Sinkhorn
half-update rebuilds (pot_m - C_nm) on the TensorEngine from rank-5 factor
matrices (-C = x.y - 0.5|x|^2 - 0.5|y|^2) plus a K=1 ones-matmul that
broadcasts the opposite potential (held row-major in a [16, 128] tile produced
by a PE transpose) into the same PSUM accumulation group.  The ScalarEngine
then runs a single fused exp+accumulate pass per [128, 2048] row-tile with the
incremental row-max upper bound U as a per-partition bias, exactly mirroring
the log-domain update

    f_n = -eps*(logw + log S_n) - U_n,   S_n = sum_m exp((g_m - C_nm - U_n)/eps)

of the reference's eps-scaled schedule (14 steps, geomspace(32, 1e-6)).
Between half-updates only [128, 16]-shaped work remains (Ln, the potential
combine, the bound refresh L/U and the max-delta via gpsimd partition
all-reduce), so the ACT engine's exp throughput is the roofline.

argmin_m(2C - g) runs the same PE pipeline with doubled x-factors, a fused
DVE copy+max (tensor_tensor_reduce) and max_index per row-tile; x0[idx] is
gathered with indirect DMA and the conditioned per-point MLP runs in
transposed [feature, point] layout on the PE.  Row layout is n = 128*c + p
(block) throughout.
"""

import os
from contextlib import ExitStack

import numpy as np

# Debug/bisect switches (default = fast path)
OPT_INPLACE_ACT = os.environ.get("K_INPLACE_ACT", "1") == "1"
OPT_PE_TRANSPOSE = os.environ.get("K_PE_TRANSPOSE", "1") == "1"
OPT_SB2SB_DMA = os.environ.get("K_SB2SB_DMA", "1") == "1"
OPT_ACT_SPAN = int(os.environ.get("K_ACT_SPAN", "2048"))
OPT_PAR = os.environ.get("K_PAR", "1") == "1"      # gpsimd partition_all_reduce
# tensor_tensor_reduce wedges the device (NRT unrecoverable) — keep off.
OPT_TTR = os.environ.get("K_TTR", "0") == "1"

import concourse.bass as bass
import concourse.bacc as bacc
import concourse.bass_isa as bass_isa
import concourse.tile as tile
from concourse import mybir
from concourse.bass_utils import run_bass_kernel_spmd
from concourse.masks import make_identity

P = 128
N = 2048
NT = N // P          # 16 row tiles
D = 3
H = 256
NCORES = 8
QW = 512
F32 = mybir.dt.float32
U32 = mybir.dt.uint32

EPS_LIST = np.geomspace(32.0, 0.001 ** 2, 14).astype(np.float32)
LOG_N = float(np.log(np.float64(N)))
LOGW = float(-LOG_N)
NEG_BIG = -3.0e38

AF = mybir.ActivationFunctionType
OP = mybir.AluOpType

LAST_EXEC_NS = None
LAST_RESULTS = None


def _build_bass_program():
    nc = bacc.Bacc("TRN2", num_devices=NCORES, debug=False)

    def inp(name, shape, dtype=F32):
        return nc.dram_tensor(name, list(shape), dtype, kind="ExternalInput").ap()

    lxi = inp("lxi", (6, N))          # [x0,x1,x2, 0.5|x|^2, 1, ones]  (x = noise)
    ryi = inp("ryi", (6, N))          # [y0,y1,y2, -1, -0.5|y|^2, g=0] (y = x0)
    lyi = inp("lyi", (6, N))          # [y0,y1,y2, -0.5|y|^2, 1, ones]
    rxi = inp("rxi", (6, N))          # [x0,x1,x2, 1, -0.5|x|^2, f=0]
    lxai = inp("lxai", (6, N))        # [2*factors, ones] (argmin needs g - 2C)
    x0g = inp("x0g", (N, D))          # gather source (x0 rows)
    noise_r = inp("noise_r", (P, D * NT))   # noise[128c+p] at [p, 3c:3c+3]
    tnt48 = inp("tnt48", (P, D * NT))  # t*noise, same layout
    omt128 = inp("omt128", (P, 1))    # (1 - t) broadcast
    w1aug = inp("w1aug", (4, H))      # W1 rows + (t*Wt + b1)
    w2r = inp("w2r", (P, 2 * D))      # W2 reshaped [128, 2*3]
    b2c = inp("b2c", (D, 1))

    vpt_out = nc.dram_tensor("vpt_out", [D, N], F32, kind="ExternalOutput").ap()
    v_out = nc.dram_tensor("v_out", [P, D * NT], F32, kind="ExternalOutput").ap()
    idx_out = nc.dram_tensor("idx_out", [P, NT], U32, kind="ExternalOutput").ap()
    pot_dram = (nc.dram_tensor("pot_f", [N], F32, kind="Internal").ap(),
                nc.dram_tensor("pot_g", [N], F32, kind="Internal").ap())

    with tile.TileContext(nc) as tc:
        with ExitStack() as ctx:
            _body(ctx, tc, lxi, ryi, lyi, rxi, lxai, x0g, noise_r, tnt48,
                  omt128, w1aug, w2r, b2c, vpt_out, v_out, idx_out, pot_dram)
    nc.compile()
    return nc


def _body(ctx, tc, lxi, ryi, lyi, rxi, lxai, x0g, noise_r, tnt48,
          omt128, w1aug, w2r, b2c, vpt_out, v_out, idx_out, pot_dram):
    nc = tc.nc
    pot_dram_f, pot_dram_g = pot_dram

    const = ctx.enter_context(tc.tile_pool(name="const", bufs=1))
    stat = ctx.enter_context(tc.tile_pool(name="stat", bufs=1))
    small = ctx.enter_context(tc.tile_pool(name="small", bufs=2))
    tvp = ctx.enter_context(tc.tile_pool(name="tvp", bufs=2))
    ps = ctx.enter_context(tc.tile_pool(name="ps", bufs=2, space="PSUM"))

    # ---- static inputs ----
    # Factor matrices live on partitions 32..37 (matmul base-partition rule);
    # row 37 of the rhs-side tiles holds the opposite potential, rewritten by
    # a 16-descriptor SBUF->SBUF DMA each half-update.
    B0 = 32           # base partition
    K6 = 6
    lx = stat.tile([B0 + K6, N], F32, tag="lx")
    ry = stat.tile([B0 + K6, N], F32, tag="ry")
    ly = stat.tile([B0 + K6, N], F32, tag="ly")
    rx = stat.tile([B0 + K6, N], F32, tag="rx")
    lxa = stat.tile([B0 + K6, N], F32, tag="lxa")
    nc.sync.dma_start(out=lx[B0:B0 + K6, :], in_=lxi[:])
    nc.sync.dma_start(out=ry[B0:B0 + K6, :], in_=ryi[:])
    nc.gpsimd.dma_start(out=ly[B0:B0 + K6, :], in_=lyi[:])
    nc.gpsimd.dma_start(out=rx[B0:B0 + K6, :], in_=rxi[:])
    nc.gpsimd.dma_start(out=lxa[B0:B0 + K6, :], in_=lxai[:])

    ident = const.tile([P, P], F32, tag="ident")
    make_identity(nc, ident[:])

    # Sinkhorn state (column layout [128, 16]; col c holds n/m = 128c+p)
    fstage = const.tile([NT, P], F32, tag="fstage")   # f, row-major [16, 128]
    gstage = const.tile([NT, P], F32, tag="gstage")   # g, row-major
    S2f = const.tile([P, NT], F32, tag="S2f")
    S2g = const.tile([P, NT], F32, tag="S2g")
    S4 = const.tile([P, NT * 4], F32, tag="S4")
    fAB = [const.tile([P, NT], F32, tag=f"f{i}", name=f"f{i}") for i in range(2)]
    gAB = [const.tile([P, NT], F32, tag=f"g{i}", name=f"g{i}") for i in range(2)]
    Uf = const.tile([P, NT], F32, tag="Uf")
    Ug = const.tile([P, NT], F32, tag="Ug")
    Lf = const.tile([P, NT], F32, tag="Lf")
    Lg = const.tile([P, NT], F32, tag="Lg")
    nUf = const.tile([P, NT], F32, tag="nUf")
    nUg = const.tile([P, NT], F32, tag="nUg")
    d_dump = const.tile([P, NT], F32, tag="d_dump")
    maxd_p = const.tile([P, 1], F32, tag="maxd_p")
    maxd_a = const.tile([P, 1], F32, tag="maxd_a")
    dmax1 = const.tile([1, 1], F32, tag="dmax1")
    ones1 = const.tile([1, P], F32, tag="ones1")
    nc.vector.memset(ones1[:], 1.0)
    m8 = const.tile([P, 8], F32, tag="m8")
    idx_buf = const.tile([P, 8 * NT], U32, tag="idx_buf")
    zeros_t = const.tile([P, N], F32, tag="zeros_t")

    for t_ in (Uf, Lg, fAB[1], gAB[1]):
        nc.vector.memset(t_[:], 0.0)
    nc.vector.memset(m8[:], NEG_BIG)
    nc.vector.memset(zeros_t[:], 0.0)

    def mm_tile(pm, lhs_mat, c, rhs_mat):
        """One [128, 2048] row-tile of (pot - C): 4 K=6 matmuls (5 factor
        rows + opposite-potential row riding along as rhs row 37)."""
        for q in range(4):
            nc.tensor.matmul(
                out=pm[:, QW * q:QW * (q + 1)],
                lhsT=lhs_mat[B0:B0 + K6, P * c:P * (c + 1)],
                rhs=rhs_mat[B0:B0 + K6, QW * q:QW * (q + 1)],
                start=True, stop=True,
            )

    def half_update(it, side):
        eps = float(EPS_LIST[it])
        inv_eps = float(1.0 / np.float64(eps))
        neg_eps = float(-np.float64(eps))
        last = (it == len(EPS_LIST) - 1 and side == "g")
        if side == "f":
            lhs_mat, rhs_mat = lx, ry
            U_cur, nU_cur, S2 = Uf, nUf, S2f
            pot_new, pot_prev = fAB[it % 2], fAB[1 - it % 2]
            stage_out, pot_dst, L_cur = fstage, rx, Lf
            U_oth, L_oth = Ug, Lg
        else:
            lhs_mat, rhs_mat = ly, rx
            U_cur, nU_cur, S2 = Ug, nUg, S2g
            pot_new, pot_prev = gAB[it % 2], gAB[1 - it % 2]
            stage_out, pot_dst, L_cur = gstage, ry, Lg
            U_oth, L_oth = Uf, Lf

        nc.vector.tensor_scalar(out=nU_cur[:], in0=U_cur[:],
                                scalar1=-inv_eps, scalar2=None, op0=OP.mult)
        for c in range(NT):
            pm = ps.tile([P, N], F32, tag="mm", name="pm")
            mm_tile(pm, lhs_mat, c, rhs_mat)
            eout = pm if OPT_INPLACE_ACT else tvp.tile([P, N], F32, tag="tv",
                                                       name="eout")
            if OPT_ACT_SPAN >= N:
                nc.scalar.activation(
                    out=eout[:], in_=pm[:], func=AF.Exp,
                    bias=nU_cur[:, c:c + 1],
                    scale=inv_eps, accum_out=S2[:, c:c + 1])
            else:
                for q0 in range(0, N, OPT_ACT_SPAN):
                    qi = q0 // OPT_ACT_SPAN
                    nc.scalar.activation(
                        out=eout[:, q0:q0 + OPT_ACT_SPAN],
                        in_=pm[:, q0:q0 + OPT_ACT_SPAN], func=AF.Exp,
                        bias=nU_cur[:, c:c + 1],
                        scale=inv_eps,
                        accum_out=S4[:, (N // OPT_ACT_SPAN) * c + qi:
                                     (N // OPT_ACT_SPAN) * c + qi + 1])
                nc.vector.tensor_reduce(
                    out=S2[:, c:c + 1],
                    in_=S4[:, (N // OPT_ACT_SPAN) * c:(N // OPT_ACT_SPAN) * (c + 1)],
                    axis=mybir.AxisListType.X, op=OP.add)

        # pot = -eps*(log S + logw) - U ; flatten to row-major via PE
        # transpose + DVE copy + 16-descriptor SBUF->SBUF DMA into row 37
        lnS = small.tile([P, NT], F32, tag="lnS", name="lnS")
        nc.scalar.activation(out=lnS[:], in_=S2[:], func=AF.Ln, bias=0.0, scale=1.0)
        half = small.tile([P, NT], F32, tag="half", name="half")
        nc.vector.tensor_scalar(out=half[:], in0=lnS[:], scalar1=LOGW,
                                scalar2=neg_eps, op0=OP.add, op1=OP.mult)
        nc.vector.tensor_tensor(out=pot_new[:], in0=half[:], in1=U_cur[:],
                                op=OP.subtract)
        tr = ps.tile([NT, P], F32, tag="mm", name="tr")
        if OPT_PE_TRANSPOSE:
            nc.tensor.transpose(tr[:], pot_new[:], ident[:])
        else:
            nc.tensor.matmul(out=tr[:], lhsT=pot_new[:], rhs=ident[:],
                             start=True, stop=True)
        nc.vector.tensor_copy(out=stage_out[:], in_=tr[:])
        if OPT_SB2SB_DMA:
            nc.sync.dma_start(out=pot_dst[B0 + K6 - 1:B0 + K6, :],
                              in_=stage_out[:])
        else:
            pd = pot_dram_f if side == "f" else pot_dram_g
            nc.sync.dma_start(out=pd[:], in_=stage_out[:])
            nc.sync.dma_start(out=pot_dst[B0 + K6 - 1:B0 + K6, :], in_=pd[:])

        if not last:
            # bound refresh: L = -(pot + eps*logw); maxd = max(pot - prev);
            # U_other = L_other + maxd
            nc.vector.tensor_scalar(out=L_cur[:], in0=pot_new[:],
                                    scalar1=float(np.float64(eps) * LOGW),
                                    scalar2=-1.0, op0=OP.add, op1=OP.mult)
            if OPT_TTR:
                nc.vector.tensor_tensor_reduce(
                    out=d_dump[:], in0=pot_new[:], in1=pot_prev[:], scale=1.0,
                    scalar=NEG_BIG, op0=OP.subtract, op1=OP.max,
                    accum_out=maxd_p[:])
            else:
                nc.vector.tensor_tensor(out=d_dump[:], in0=pot_new[:],
                                        in1=pot_prev[:], op=OP.subtract)
                nc.vector.tensor_reduce(out=maxd_p[:], in_=d_dump[:],
                                        axis=mybir.AxisListType.X, op=OP.max)
            if OPT_PAR:
                nc.gpsimd.partition_all_reduce(
                    out_ap=maxd_a[:], in_ap=maxd_p[:], channels=P,
                    reduce_op=bass_isa.ReduceOp.max)
            else:
                nc.gpsimd.tensor_reduce(out=dmax1[:], in_=maxd_p[:],
                                        axis=mybir.AxisListType.C, op=OP.max)
                sc = ps.tile([P, 2], F32, tag="mm", name="sc")
                nc.tensor.matmul(out=sc[:, 0:1], lhsT=ones1[:, :],
                                 rhs=dmax1[:, 0:1], start=True, stop=True)
                nc.scalar.copy(out=maxd_a[:], in_=sc[:, 0:1])
            nc.vector.tensor_scalar(out=U_oth[:], in0=L_oth[:],
                                    scalar1=maxd_a[:, 0:1], scalar2=None,
                                    op0=OP.add)

    # ---- Sinkhorn ----
    for it in range(len(EPS_LIST)):
        half_update(it, "f")
        half_update(it, "g")

    # ---- argmin_m(2C - g) = argmax_m(g - 2C), gather overlapped ----
    mlp = ctx.enter_context(tc.tile_pool(name="mlp", bufs=1))
    x0a = mlp.tile([P, D * NT], F32, tag="x0a")
    for c in range(NT):
        pm = ps.tile([P, N], F32, tag="mm", name="pma")
        mm_tile(pm, lxa, c, ry)
        tmpv = tvp.tile([P, N], F32, tag="tv", name="tmpv")
        if OPT_TTR:
            nc.vector.tensor_tensor_reduce(
                out=tmpv[:], in0=pm[:], in1=zeros_t[:], scale=1.0,
                scalar=NEG_BIG, op0=OP.add, op1=OP.max, accum_out=m8[:, 0:1])
        else:
            nc.scalar.copy(out=tmpv[:], in_=pm[:])
            nc.vector.max(out=m8[:], in_=tmpv[:])
        nc.vector.max_index(
            out=idx_buf[:, 8 * c:8 * (c + 1)],
            in_max=m8[:],
            in_values=tmpv[:],
        )
        nc.gpsimd.indirect_dma_start(
            out=x0a[:, D * c:D * (c + 1)],
            out_offset=None,
            in_=x0g[:],
            in_offset=bass.IndirectOffsetOnAxis(ap=idx_buf[:, 8 * c:8 * c + 1], axis=0),
        )
    nc.sync.dma_start(out=idx_out[:], in_=idx_buf[:, 0::8])

    # ---- MLP ----
    noise_sb = mlp.tile([P, D * NT], F32, tag="noise")
    nc.sync.dma_start(out=noise_sb[:], in_=noise_r[:])
    v_sb = mlp.tile([P, D * NT], F32, tag="v")
    nc.vector.tensor_tensor(out=v_sb[:], in0=noise_sb[:], in1=x0a[:],
                            op=OP.subtract)
    nc.sync.dma_start(out=v_out[:], in_=v_sb[:])

    # x_t in point-major [128, 48], then 16 PE transposes -> xtT [4, 2048]
    tnt_sb = mlp.tile([P, D * NT], F32, tag="tnt_sb")
    nc.sync.dma_start(out=tnt_sb[:], in_=tnt48[:])
    omt_sb = mlp.tile([P, 1], F32, tag="omt")
    nc.sync.dma_start(out=omt_sb[:], in_=omt128[:])
    xt48 = mlp.tile([P, D * NT], F32, tag="xt48")
    nc.vector.scalar_tensor_tensor(
        out=xt48[:],
        in0=x0a[:],
        scalar=omt_sb[:, 0:1],
        in1=tnt_sb[:],
        op0=OP.mult, op1=OP.add,
    )
    trm = ps.tile([D, N], F32, tag="mm", name="trm")
    for c in range(NT):
        if OPT_PE_TRANSPOSE:
            nc.tensor.transpose(trm[0:D, P * c:P * (c + 1)],
                                xt48[:, D * c:D * (c + 1)], ident[:])
        else:
            nc.tensor.matmul(out=trm[0:D, P * c:P * (c + 1)],
                             lhsT=xt48[:, D * c:D * (c + 1)], rhs=ident[:],
                             start=True, stop=True)
    xtT = mlp.tile([4, N], F32, tag="xtT")
    nc.vector.memset(xtT[:], 1.0)
    nc.scalar.copy(out=xtT[0:D, :], in_=trm[0:D, :])

    w1_sb = mlp.tile([4, H], F32, tag="w1")
    nc.sync.dma_start(out=w1_sb[:], in_=w1aug[:])
    w2_sb = mlp.tile([P, 2 * D], F32, tag="w2")
    nc.sync.dma_start(out=w2_sb[:], in_=w2r[:])
    b2_sb = mlp.tile([D, 1], F32, tag="b2")
    nc.sync.dma_start(out=b2_sb[:], in_=b2c[:])

    # h^T = relu(W1aug^T @ xt_aug^T) -> two [128, 2048] tiles
    h_tiles = []
    for c2 in range(2):
        pmh = ps.tile([P, N], F32, tag="mm", name="pmh")
        for q in range(4):
            nc.tensor.matmul(
                out=pmh[:, QW * q:QW * (q + 1)],
                lhsT=w1_sb[:, P * c2:P * (c2 + 1)],
                rhs=xtT[:, QW * q:QW * (q + 1)],
                start=True, stop=True,
            )
        ht = tvp.tile([P, N], F32, tag="tv", name="ht")
        nc.scalar.activation(out=ht[:], in_=pmh[:], func=AF.Relu,
                             bias=0.0, scale=1.0)
        h_tiles.append(ht)

    # v_pred^T = W2^T @ h^T + b2 -> [3, 2048]
    pmv = ps.tile([P, N], F32, tag="mm", name="pmv")
    for q in range(4):
        for c2 in range(2):
            nc.tensor.matmul(
                out=pmv[0:D, QW * q:QW * (q + 1)],
                lhsT=w2_sb[:, D * c2:D * (c2 + 1)],
                rhs=h_tiles[c2][:, QW * q:QW * (q + 1)],
                start=(c2 == 0), stop=(c2 == 1),
            )
    vpt_sb = mlp.tile([D, N], F32, tag="vpt_sb")
    nc.scalar.activation(out=vpt_sb[:], in_=pmv[0:D, :], func=AF.Identity,
                         bias=b2_sb[:, 0:1], scale=1.0)
    nc.sync.dma_start(out=vpt_out[:], in_=vpt_sb[:])


_PROGRAM_CACHE = None


def _get_program():
    global _PROGRAM_CACHE
    if _PROGRAM_CACHE is None:
        _PROGRAM_CACHE = _build_bass_program()
    return _PROGRAM_CACHE


def _host_prep(cloud, noise, t, W1, Wt, b1, W2, b2):
    """Per-sample input preparation (numpy, O(N*D))."""
    B = cloud.shape[0]
    in_maps = []
    ones = np.ones(N, np.float32)
    for b in range(B):
        std = np.std(cloud[b].astype(np.float64), ddof=1)
        x0 = (cloud[b].astype(np.float64) / std).astype(np.float32)   # y
        x = np.ascontiguousarray(noise[b].astype(np.float32))          # x
        tb = np.float32(t[b])

        xn2 = (0.5 * np.sum(x.astype(np.float64) ** 2, axis=1)).astype(np.float32)
        yn2 = (0.5 * np.sum(x0.astype(np.float64) ** 2, axis=1)).astype(np.float32)
        zero = np.zeros(N, np.float32)
        lxi = np.stack([x[:, 0], x[:, 1], x[:, 2], xn2, ones, ones])
        ryi = np.stack([x0[:, 0], x0[:, 1], x0[:, 2], -ones, -yn2, zero])
        lyi = np.stack([x0[:, 0], x0[:, 1], x0[:, 2], -yn2, ones, ones])
        rxi = np.stack([x[:, 0], x[:, 1], x[:, 2], ones, -xn2, zero])
        lxai = np.stack([2 * x[:, 0], 2 * x[:, 1], 2 * x[:, 2],
                         2 * xn2, 2 * ones, ones])

        noise_r = x.reshape(NT, P, D).transpose(1, 0, 2).reshape(P, D * NT)
        tnt48 = (tb * x).reshape(NT, P, D).transpose(1, 0, 2).reshape(P, D * NT)
        omt128 = np.full((P, 1), np.float32(1.0) - tb, np.float32)
        w1aug = np.concatenate([W1.astype(np.float32),
                                (tb * Wt + b1).astype(np.float32)[None, :]], axis=0)
        w2r = W2.astype(np.float32).reshape(2, P, D).transpose(1, 0, 2).reshape(P, 2 * D)
        b2c = b2.astype(np.float32).reshape(D, 1)

        in_maps.append({
            "lxi": np.ascontiguousarray(lxi, np.float32),
            "ryi": np.ascontiguousarray(ryi, np.float32),
            "lyi": np.ascontiguousarray(lyi, np.float32),
            "rxi": np.ascontiguousarray(rxi, np.float32),
            "lxai": np.ascontiguousarray(lxai, np.float32),
            "x0g": np.ascontiguousarray(x0),
            "noise_r": np.ascontiguousarray(noise_r),
            "tnt48": np.ascontiguousarray(tnt48),
            "omt128": omt128,
            "w1aug": np.ascontiguousarray(w1aug),
            "w2r": np.ascontiguousarray(w2r),
            "b2c": b2c,
        })
    return in_maps


def _unshard(results, B):
    v_pred = np.empty((B, N, D), np.float32)
    v = np.empty((B, N, D), np.float32)
    for b in range(B):
        r = results[b]
        v[b] = r["v_out"].reshape(P, NT, D).transpose(1, 0, 2).reshape(N, D)
        v_pred[b] = r["vpt_out"].T
    return v_pred, v


def kernel(cloud, noise, t, W1, Wt, b1, W2, b2, _trace=False):
    global LAST_EXEC_NS, LAST_RESULTS
    cloud = np.asarray(cloud, np.float32)
    noise = np.asarray(noise, np.float32)
    t = np.asarray(t, np.float32)
    W1 = np.asarray(W1, np.float32)
    Wt = np.asarray(Wt, np.float32)
    b1 = np.asarray(b1, np.float32)
    W2 = np.asarray(W2, np.float32)
    b2 = np.asarray(b2, np.float32)

    nc = _get_program()
    in_maps = _host_prep(cloud, noise, t, W1, Wt, b1, W2, b2)
    res = run_bass_kernel_spmd(nc, in_maps, core_ids=list(range(NCORES)),
                               trace=_trace)
    LAST_EXEC_NS = res.exec_time_ns
    LAST_RESULTS = res
    return _unshard(res.results, cloud.shape[0])


# revision 33
# speedup vs baseline: 1.7377x; 1.7377x over previous
"""Trainium2 Bass kernel for nn_DiffusionModel (Sinkhorn OT assignment + per-point MLP).

Data-parallel over the batch: each of the 8 NeuronCores processes one sample
(B=8).  Per core:

  1. Build the cost matrix C = 0.5*||noise_n - x0_m||^2 [2048 x 2048] on the
     TensorEngine from rank-5 factor matrices; keep C (row layout) resident in
     SBUF and stage C^T to a DRAM scratch tensor.  Row chunks are interleaved:
     tile j holds rows {n : n % 16 == j} (partition p <-> n = 16p + j), which
     lets the per-chunk potential columns [128, 16] flatten to an n-ordered
     [2048] vector with one contiguous DMA.

  2. 14 epsilon-scaled log-domain Sinkhorn iterations.  Each potential update
     is two fused full-matrix passes per [128, 2048] tile:
       DVE  tensor_tensor_reduce: tmp = (pot_bcast - C) * (-1/eps),
                                  acc = min_m(tmp)   (= -rowmax/eps)
       ACT  activation(Exp):      S = sum_m exp(-tmp + acc)   (fused accum)
     so   f = eps*acc - eps*(log S + log w).  The updated potential is
     flattened via a DRAM bounce and re-broadcast across partitions into a
     [128, 2048] PSUM tile with K=1 ones-matmuls.  The g-update streams C^T
     tiles back from DRAM (double buffered) since both orientations do not
     fit in SBUF in fp32.

  3. argmin_m(2C - g) via one more TTR pass (max accum) + max_index.

  4. Gather x0[idx] with indirect DMA; v = noise - x0a in row layout; the
     conditioned MLP runs in transposed [feature, point] layout on the PE.
"""

from contextlib import ExitStack

import numpy as np

import concourse.bass as bass
import concourse.bacc as bacc
import concourse.bass_isa as bass_isa
import concourse.tile as tile
from concourse import mybir
from concourse.bass_utils import run_bass_kernel_spmd
from concourse.masks import make_identity

P = 128
N = 2048
NT = N // P          # 16 tiles per matrix orientation
D = 3
H = 256
NCORES = 8
QW = 512
F32 = mybir.dt.float32
U32 = mybir.dt.uint32

EPS_LIST = np.geomspace(32.0, 0.001 ** 2, 14).astype(np.float32)
LOG_N = float(np.log(np.float64(N)))
POS_BIG = 3.0e38
NEG_BIG = -3.0e38

AF = mybir.ActivationFunctionType
OP = mybir.AluOpType
AX = mybir.AxisListType

LAST_EXEC_NS = None
LAST_RESULTS = None


def _bcast_dma(nc, bcast_sb, pot_cols, pot_dram):
    """Flatten [128, 16] -> DRAM [2048] (n = 16p + j order), then one
    partition-broadcast read: bcast_sb[p, m] = pot_dram[m] for all p
    (DRAM source APs may lead with a stride-0 replication dim)."""
    nc.sync.dma_start(out=pot_dram[:], in_=pot_cols[:])
    src_ap = bass.AP(tensor=pot_dram.tensor, offset=pot_dram.offset,
                     ap=[[0, P]] + [list(d) for d in pot_dram.ap])
    nc.sync.dma_start(out=bcast_sb[:], in_=src_ap)


def _potential_update(nc, tmp_pool, small, mats, bcast_ps, eps, logw, it,
                      S_cols, U_cols, L_cols, prev_cols, maxd_in, maxd_out,
                      pot_cols, pot_dram, ones1, sc_ps, sc_col,
                      dmax1, dmax_p, per_tile_hook=None):
    """One Sinkhorn half-update using an incremental upper bound U on the
    row-max (log-sum-exp is shift invariant; slack only costs fp underflow,
    validated < 30*eps on this problem).

    mats: 16 [128, 2048] cost tiles (C or C^T).
    U_cols/L_cols/prev_cols: bound state; maxd_in is the broadcast potential's
    max-delta, maxd_out receives this potential's max-delta.
    S_cols: [128, 16] accumulator for the exp sums.
    Writes the new potential to pot_cols, flattens to pot_row, re-broadcasts
    into bcast_ps, and refreshes the bound state for the *other* orientation.
    """
    inv_eps = float(1.0 / np.float64(eps))
    neg_eps = float(-np.float64(eps))

    if it > 0:
        # U = L_prev + max-delta of the other potential
        nc.vector.tensor_scalar(out=U_cols[:], in0=L_cols[:],
                                scalar1=maxd_in[:, 0:1], scalar2=None,
                                op0=OP.add)
    nUf = small.tile([P, NT], F32, tag="nuf", name="nuf")
    nc.vector.tensor_scalar(out=nUf[:], in0=U_cols[:], scalar1=-inv_eps,
                            scalar2=None, op0=OP.mult)

    for j in range(NT):
        tmp = tmp_pool.tile([P, N], F32, tag="tmp", name="tmp")
        nc.vector.tensor_tensor(out=tmp[:], in0=bcast_ps[:, :],
                                in1=mats[j][:], op=OP.subtract)
        nc.scalar.activation(out=tmp[:], in_=tmp[:], func=AF.Exp,
                             bias=nUf[:, j:j + 1], scale=inv_eps,
                             accum_out=S_cols[:, j:j + 1])
        if per_tile_hook is not None:
            per_tile_hook(j)

    # pot = -eps*(log S + logw) - U
    logs = small.tile([P, NT], F32, tag="logs", name="logs")
    nc.scalar.activation(out=logs[:], in_=S_cols[:], func=AF.Ln,
                         bias=0.0, scale=1.0)
    half = small.tile([P, NT], F32, tag="half", name="half")
    nc.vector.tensor_scalar(out=half[:], in0=logs[:], scalar1=logw,
                            scalar2=neg_eps, op0=OP.add, op1=OP.mult)
    nc.vector.tensor_tensor(out=pot_cols[:], in0=half[:], in1=U_cols[:],
                            op=OP.subtract)

    # bound refresh: L = -pot - eps*logw ; maxd = max(pot - prev); prev = pot
    nc.vector.tensor_scalar(out=L_cols[:], in0=pot_cols[:],
                            scalar1=float(np.float64(eps) * logw), scalar2=-1.0,
                            op0=OP.add, op1=OP.mult)
    d_cols = small.tile([P, NT], F32, tag="d_cols", name="d_cols")
    nc.vector.tensor_tensor(out=d_cols[:], in0=pot_cols[:], in1=prev_cols[:],
                            op=OP.subtract)
    nc.vector.tensor_copy(out=prev_cols[:], in_=pot_cols[:])
    nc.vector.tensor_reduce(out=dmax_p[:], in_=d_cols[:], axis=AX.X, op=OP.max)
    # all-partition max in one gpsimd op (replaces the slow C-axis reduce +
    # ones-matmul broadcast + copy chain on the inter-update critical path)
    nc.gpsimd.partition_all_reduce(out_ap=maxd_out[:], in_ap=dmax_p[:],
                                   channels=P, reduce_op=bass_isa.ReduceOp.max)

    # flatten + partition-broadcast via DRAM (bcast_sb[p, m] = pot_m)
    _bcast_dma(nc, bcast_ps, pot_cols[:], pot_dram)


def _build_bass_program():
    nc = bacc.Bacc("TRN2", num_devices=NCORES, debug=False)

    def inp(name, shape, dtype=F32):
        return nc.dram_tensor(name, list(shape), dtype, kind="ExternalInput").ap()

    xf = inp("xf", (5, N))            # rows: x0,x1,x2, 0.5|x|^2, 1     (x = noise)
    yf = inp("yf", (5, N))            # rows: -y0,-y1,-y2, 1, 0.5|y|^2  (y = x0)
    x0g = inp("x0g", (N, D))          # gather source (x0 rows)
    noise_r = inp("noise_r", (P, D * NT))   # noise[16p+j] at [p, 3j:3j+3]
    tnt = inp("tnt", (D, N))          # t*noise^T (n-order columns)
    omt3 = inp("omt3", (D, 1))        # (1 - t)
    w1aug = inp("w1aug", (4, H))      # W1 rows + (t*Wt + b1)
    w2r = inp("w2r", (P, 2 * D))      # W2 reshaped [128, 2*3]
    b2c = inp("b2c", (D, 1))

    vpt_out = nc.dram_tensor("vpt_out", [D, N], F32, kind="ExternalOutput").ap()
    v_out = nc.dram_tensor("v_out", [P, D * NT], F32, kind="ExternalOutput").ap()
    idx_out = nc.dram_tensor("idx_out", [P, NT], U32, kind="ExternalOutput").ap()
    ct_dram = nc.dram_tensor("ct_scratch", [NT, P, N], F32, kind="Internal").ap()
    f_dram = nc.dram_tensor("f_scratch", [N], F32, kind="Internal").ap()
    g_dram = nc.dram_tensor("g_scratch", [N], F32, kind="Internal").ap()
    xa_dram = nc.dram_tensor("xa_scratch", [N, D], F32, kind="Internal").ap()

    with tile.TileContext(nc) as tc:
        with ExitStack() as ctx:
            _body(ctx, tc, xf, yf, x0g, noise_r, tnt, omt3, w1aug, w2r, b2c,
                  vpt_out, v_out, idx_out, ct_dram, f_dram, g_dram, xa_dram)
    nc.compile()
    return nc


def _body(ctx, tc, xf, yf, x0g, noise_r, tnt, omt3, w1aug, w2r, b2c,
          vpt_out, v_out, idx_out, ct_dram, f_dram, g_dram, xa_dram):
    nc = tc.nc

    const = ctx.enter_context(tc.tile_pool(name="const", bufs=1))
    cmat = ctx.enter_context(tc.tile_pool(name="cmat", bufs=1))
    ring = ctx.enter_context(tc.tile_pool(name="ring", bufs=5))
    tmp_pool = ctx.enter_context(tc.tile_pool(name="tmp", bufs=3))
    small = ctx.enter_context(tc.tile_pool(name="small", bufs=1))
    ps_sc = ctx.enter_context(tc.tile_pool(name="pssc", bufs=1, space="PSUM"))
    ps_mm = ctx.enter_context(tc.tile_pool(name="psc", bufs=2, space="PSUM"))

    # ---- constants / inputs to SBUF ----
    # factor matrices live in ring slots; they are fully consumed by the end
    # of iteration 0's f-update (C^T build hook), after which the slots
    # recycle into the C^T streaming ring.
    xf_sb = ring.tile([5, N], F32, tag="ring", name="xf_sb")
    yf_sb = ring.tile([5, N], F32, tag="ring", name="yf_sb")
    nc.sync.dma_start(out=xf_sb[:], in_=xf[:])
    nc.sync.dma_start(out=yf_sb[:], in_=yf[:])

    ones1 = const.tile([1, P], F32, tag="ones1")
    nc.vector.memset(ones1[:], 1.0)

    S_f = const.tile([P, NT], F32, tag="S_f")
    S_g = const.tile([P, NT], F32, tag="S_g")
    f_cols = const.tile([P, NT], F32, tag="f_cols")
    g_cols = const.tile([P, NT], F32, tag="g_cols")
    U_f = const.tile([P, NT], F32, tag="U_f")
    U_g = const.tile([P, NT], F32, tag="U_g")
    L_f = const.tile([P, NT], F32, tag="L_f")
    L_g = const.tile([P, NT], F32, tag="L_g")
    fprev = const.tile([P, NT], F32, tag="fprev")
    gprev = const.tile([P, NT], F32, tag="gprev")
    maxdf = const.tile([P, 1], F32, tag="maxdf")
    maxdg = const.tile([P, 1], F32, tag="maxdg")
    dmax_p = const.tile([P, 1], F32, tag="dmax_p")
    dmax1 = const.tile([1, 1], F32, tag="dmax1")
    idx_buf = const.tile([P, 8 * NT], U32, tag="idx_buf")
    for t_ in (U_f, L_g, fprev, gprev):
        nc.vector.memset(t_[:], 0.0)

    bcast_ps = const.tile([P, N], F32, tag="bcast")
    sc_ps = ps_sc.tile([P, 2], F32, tag="sc")

    # ---- phase 1: build C (SBUF resident, interleaved rows) and C^T (to DRAM) ----
    c_tiles = []
    for j in range(NT):
        c_tiles.append(cmat.tile([P, N], F32, tag=f"c{j}", name=f"c{j}"))
    for j in range(NT):
        # C tile j: rows n = 16p + j; lhsT = xf[:, j::16] (strided), rhs = yf
        for q in range(4):
            mm = ps_mm.tile([P, QW], F32, tag="mm", name="mm")
            nc.tensor.matmul(
                out=mm[:],
                lhsT=xf_sb[:, j::NT],
                rhs=yf_sb[:, q * QW:(q + 1) * QW],
                start=True, stop=True,
            )
            if q % 2 == 0:
                nc.scalar.copy(out=c_tiles[j][:, q * QW:(q + 1) * QW], in_=mm[:])
            else:
                nc.vector.tensor_copy(out=c_tiles[j][:, q * QW:(q + 1) * QW], in_=mm[:])

    # ---- phase 2: Sinkhorn ----
    logw = float(-LOG_N)
    # initial g = 0
    nc.vector.memset(g_cols[:], 0.0)
    nc.vector.memset(bcast_ps[:], 0.0)

    def _ct_build_tile(j):
        # C^T tile j: rows m = 16p + j; lhsT = yf[:, j::16], rhs = xf.
        # Emitted inside iteration 0's f-update so the PE matmuls and
        # PSUM->SBUF copies overlap the DVE/ACT passes; DMA-out goes on the
        # gpsimd (SWDGE) queue so it cannot head-of-line block the sync-queue
        # ring streaming of the g-updates.
        stage = tmp_pool.tile([P, N], F32, tag="tmp", name="stage")
        for q in range(4):
            mm = ps_mm.tile([P, QW], F32, tag="mm", name="mm")
            nc.tensor.matmul(
                out=mm[:],
                lhsT=yf_sb[:, j::NT],
                rhs=xf_sb[:, q * QW:(q + 1) * QW],
                start=True, stop=True,
            )
            if q % 2 == 0:
                nc.scalar.copy(out=stage[:, q * QW:(q + 1) * QW], in_=mm[:])
            else:
                nc.vector.tensor_copy(out=stage[:, q * QW:(q + 1) * QW], in_=mm[:])
        nc.gpsimd.dma_start(out=ct_dram[j], in_=stage[:])

    for it, eps in enumerate(EPS_LIST):
        eps = float(eps)
        # f-update over resident C tiles (bcast_ps currently holds g);
        # U_f = L_f + maxdg (it=0: U_f = 0 from memset)
        _potential_update(nc, tmp_pool, small, c_tiles, bcast_ps, eps, logw, it,
                          S_f, U_f, L_f, fprev, maxdg, maxdf,
                          f_cols, f_dram, ones1, sc_ps, 0,
                          dmax1, dmax_p,
                          per_tile_hook=_ct_build_tile if it == 0 else None)
        # g-update over streamed C^T tiles (bcast_ps now holds f);
        # U_g = L_g + maxdf (it=0: L_g = 0, maxdf = max f)
        ct_ring = []
        for j in range(NT):
            rt = ring.tile([P, N], F32, tag="ring", name=f"ring{j}")
            eng = nc.sync if j % 2 == 0 else nc.gpsimd
            eng.dma_start(out=rt[:], in_=ct_dram[j])
            ct_ring.append(rt)
        _potential_update(nc, tmp_pool, small, ct_ring, bcast_ps, eps, logw, 1,
                          S_g, U_g, L_g, gprev, maxdf, maxdg,
                          g_cols, g_dram, ones1, sc_ps, 1,
                          dmax1, dmax_p)

    # ---- phase 3: argmin_m (2*C_nm - g_m), gather overlapped ----
    mlp = ctx.enter_context(tc.tile_pool(name="mlp", bufs=1))
    x0a = mlp.tile([P, D * NT], F32, tag="x0a")
    # bcast_ps already holds the final g after the last g-update
    for j in range(NT):
        tmpv = tmp_pool.tile([P, N], F32, tag="tmp", name="tmpv")
        # tmpv = g - 2C  (argmax_m = argmin_m of 2C - g)
        nc.vector.scalar_tensor_tensor(out=tmpv[:], in0=c_tiles[j][:],
                                       scalar=-2.0, in1=bcast_ps[:, :],
                                       op0=OP.mult, op1=OP.add)
        m8 = small.tile([P, 8], F32, tag="m8", name="m8")
        nc.vector.max(out=m8[:], in_=tmpv[:])
        nc.vector.max_index(
            out=idx_buf[:, 8 * j:8 * (j + 1)],
            in_max=m8[:],
            in_values=tmpv[:],
        )
        nc.gpsimd.indirect_dma_start(
            out=x0a[:, D * j:D * (j + 1)],
            out_offset=None,
            in_=x0g[:],
            in_offset=bass.IndirectOffsetOnAxis(ap=idx_buf[:, 8 * j:8 * j + 1], axis=0),
        )
    nc.sync.dma_start(out=idx_out[:], in_=idx_buf[:, 0::8])

    # ---- phase 4: MLP ----

    # v = noise - x0_aligned (row layout [128, 48]; row order n = 16p + j)
    noise_sb = mlp.tile([P, D * NT], F32, tag="noise")
    nc.sync.dma_start(out=noise_sb[:], in_=noise_r[:])
    v_sb = mlp.tile([P, D * NT], F32, tag="v")
    nc.vector.tensor_tensor(out=v_sb[:], in0=noise_sb[:], in1=x0a[:],
                            op=OP.subtract)
    nc.sync.dma_start(out=v_out[:], in_=v_sb[:])

    # x0a^T via DRAM bounce: [128, 48] rows (n = 16p+j) -> [3, 2048] (n-major)
    nc.sync.dma_start(out=xa_dram[:], in_=x0a[:])
    x0aT = tmp_pool.tile([D, N], F32, tag="tmp", name="x0aT")
    nc.sync.dma_start(out=x0aT[:], in_=xa_dram[:].rearrange("n d -> d n"))
    # x_t^T = (1-t)*x0a^T + t*noise^T with ones row -> [4, 2048]
    tnt_sb = tmp_pool.tile([D, N], F32, tag="tmp", name="tnt_sb")
    nc.sync.dma_start(out=tnt_sb[:], in_=tnt[:])
    omt_sb = mlp.tile([D, 1], F32, tag="omt")
    nc.sync.dma_start(out=omt_sb[:], in_=omt3[:])
    xtT = tmp_pool.tile([4, N], F32, tag="tmp", name="xtT")
    nc.vector.memset(xtT[:], 1.0)
    nc.vector.scalar_tensor_tensor(
        out=xtT[0:D, :],
        in0=x0aT[:],
        scalar=omt_sb[:, 0:1],
        in1=tnt_sb[:],
        op0=OP.mult, op1=OP.add,
    )

    # h^T = relu(W1aug^T @ xt_aug^T) -> two [128, 2048] tiles
    w1_sb = mlp.tile([4, H], F32, tag="w1")
    nc.sync.dma_start(out=w1_sb[:], in_=w1aug[:])
    w2_sb = mlp.tile([P, 2 * D], F32, tag="w2")
    nc.sync.dma_start(out=w2_sb[:], in_=w2r[:])
    b2_sb = mlp.tile([D, 1], F32, tag="b2")
    nc.sync.dma_start(out=b2_sb[:], in_=b2c[:])

    h_tiles = []
    for c in range(2):
        ht = ring.tile([P, N], F32, tag="ring", name=f"ht{c}")
        for q in range(4):
            hq = ps_mm.tile([P, QW], F32, tag="mm", name="hq")
            nc.tensor.matmul(
                out=hq[:],
                lhsT=w1_sb[:, c * P:(c + 1) * P],
                rhs=xtT[:, q * QW:(q + 1) * QW],
                start=True, stop=True,
            )
            nc.scalar.activation(out=ht[:, q * QW:(q + 1) * QW], in_=hq[:],
                                 func=AF.Relu, bias=0.0, scale=1.0)
        h_tiles.append(ht)

    # v_pred^T = W2^T @ h^T + b2 -> [3, 2048]
    vpt_sb = tmp_pool.tile([D, N], F32, tag="tmp", name="vpt_sb")
    for q in range(4):
        vq = ps_mm.tile([P, QW], F32, tag="mm", name="vq")
        for c in range(2):
            nc.tensor.matmul(
                out=vq[0:D, 0:QW],
                lhsT=w2_sb[:, D * c:D * (c + 1)],
                rhs=h_tiles[c][:, q * QW:(q + 1) * QW],
                start=(c == 0), stop=(c == 1),
            )
        nc.scalar.activation(out=vpt_sb[:, q * QW:(q + 1) * QW], in_=vq[0:D, 0:QW],
                             func=AF.Identity, bias=b2_sb[:, 0:1], scale=1.0)
    nc.sync.dma_start(out=vpt_out[:], in_=vpt_sb[:])


_PROGRAM_CACHE = None


def _get_program():
    global _PROGRAM_CACHE
    if _PROGRAM_CACHE is None:
        _PROGRAM_CACHE = _build_bass_program()
    return _PROGRAM_CACHE


def _host_prep(cloud, noise, t, W1, Wt, b1, W2, b2):
    """Per-sample input preparation (numpy, O(N*D))."""
    B = cloud.shape[0]
    in_maps = []
    for b in range(B):
        std = np.std(cloud[b].astype(np.float64), ddof=1)
        x0 = (cloud[b].astype(np.float64) / std).astype(np.float32)   # y
        x = np.ascontiguousarray(noise[b].astype(np.float32))          # x
        tb = np.float32(t[b])

        xn2 = 0.5 * np.sum(x.astype(np.float64) ** 2, axis=1)
        yn2 = 0.5 * np.sum(x0.astype(np.float64) ** 2, axis=1)
        xf = np.stack([x[:, 0], x[:, 1], x[:, 2],
                       xn2.astype(np.float32), np.ones(N, np.float32)]).astype(np.float32)
        yf = np.stack([-x0[:, 0], -x0[:, 1], -x0[:, 2],
                       np.ones(N, np.float32), yn2.astype(np.float32)]).astype(np.float32)

        noise_r = x.reshape(P, NT, D).reshape(P, D * NT)   # row n = 16p + j
        tnt = np.ascontiguousarray((tb * x).T)              # n-order columns
        omt3 = np.full((D, 1), np.float32(1.0) - tb, np.float32)
        w1aug = np.concatenate([W1.astype(np.float32),
                                (tb * Wt + b1).astype(np.float32)[None, :]], axis=0)
        w2r = W2.astype(np.float32).reshape(2, P, D).transpose(1, 0, 2).reshape(P, 2 * D)
        b2c = b2.astype(np.float32).reshape(D, 1)

        in_maps.append({
            "xf": np.ascontiguousarray(xf),
            "yf": np.ascontiguousarray(yf),
            "x0g": np.ascontiguousarray(x0),
            "noise_r": np.ascontiguousarray(noise_r),
            "tnt": tnt,
            "omt3": omt3,
            "w1aug": np.ascontiguousarray(w1aug),
            "w2r": np.ascontiguousarray(w2r),
            "b2c": np.ascontiguousarray(b2c),
        })
    return in_maps


def _unshard(results, B):
    v_pred = np.empty((B, N, D), np.float32)
    v = np.empty((B, N, D), np.float32)
    for b in range(B):
        r = results[b]
        v[b] = r["v_out"].reshape(P, NT, D).reshape(N, D)   # row order n = 16p+j
        v_pred[b] = r["vpt_out"].T
    return v_pred, v


def kernel(cloud, noise, t, W1, Wt, b1, W2, b2, _trace=False):
    global LAST_EXEC_NS, LAST_RESULTS
    cloud = np.asarray(cloud, np.float32)
    noise = np.asarray(noise, np.float32)
    t = np.asarray(t, np.float32)
    W1 = np.asarray(W1, np.float32)
    Wt = np.asarray(Wt, np.float32)
    b1 = np.asarray(b1, np.float32)
    W2 = np.asarray(W2, np.float32)
    b2 = np.asarray(b2, np.float32)

    nc = _get_program()
    in_maps = _host_prep(cloud, noise, t, W1, Wt, b1, W2, b2)
    res = run_bass_kernel_spmd(nc, in_maps, core_ids=list(range(NCORES)),
                               trace=_trace)
    LAST_EXEC_NS = res.exec_time_ns
    LAST_RESULTS = res
    return _unshard(res.results, cloud.shape[0])

